# revision 22
# baseline (speedup 1.0000x reference)
"""Longformer self-attention Trainium2 kernel (8-core SPMD), v2.

Sharding: core c handles batch b = c//4 and heads [3*(c%4), 3*(c%4)+3).
Each core computes its 3 heads' output transposed [192, 4096]; the host
reassembles [2, 4096, 768] (transposing back).

v2 structural changes vs the v1 baseline (285us):
  - Band PV computed transposed: out[d, q] = sum_p v[p, d] * bexp[p, q]
    with a [128, 128] stationary (v head cols 0..63 | 64 ones columns):
    the moving dim rises to 256 and the softmax denominator comes out
    REPLICATED on PSUM partitions 64..127 (each ones column sums bexp),
    so normalization is reciprocal + one tensor_tensor mul -- no
    partition broadcast needed. This replaces v1's 14 LDWEIGHTS-bound
    N=65 matmuls per (block, head) with 7 N=256 matmuls.
  - q/k/kg projections fused into one [768, 576] weight so all matmuls
    except one are full M=128; same-partition bias evictions move to
    the scalar engine (frees DVE), partition-shifted ones stay on DVE.
  - Global-token scores computed transposed via col-tiled (3 heads
    concurrent) N=512 matmuls, then moved to natural [kpos, g] layout
    with the DMA xbar transpose engine; global PV keeps the cheap
    16-col-stationary accumulation.
  - kpos<G exclusion from the band softmax implemented by zeroing
    rows 0..15 of the band stationary for chunk 0 (v AND ones columns):
    numerator and denominator contributions vanish exactly, matching
    exp(-10000 + x) == 0 in f32. The remaining band-validity masks are
    two paired [128, 512] multiplies per block.
  - Band blocks are interleaved into the projection phase (block t
    issues once kpos chunk 2t+3 exists) so the PE stream stays dense:
    the v1 trace showed the HAM clock-gate dropping to K=4/8 at ~142us
    and never recovering -- half the kernel ran at 1.2 GHz.
"""

import sys

sys.path.insert(0, "/opt/trn_rl_repo")

import numpy as np
import ml_dtypes

B, S, Dm, H, WIN, G, HD = 2, 4096, 768, 12, 256, 16, 64
HPC = 3            # heads per core
NCORES = 8
DPC = HPC * HD     # 192 output dims per core
NB = S // WIN      # 16 query blocks
NKC = S // 128     # 32 kpos chunks of 128
NST = 8            # 512-wide s-tiles
SCALE = 1.0 / 8.0  # 1/sqrt(64)

_CACHE = {}


def _chunk_range(t):
    if t == 0:
        return 2, 6
    if t == NB - 1:
        return 0, 4
    return 0, 6


def _build_masks():
    """Validity masks in transposed-score orientation [kpos_local p, q r].

    maskL covers window chunks c=0,1 (left boundary), maskR covers
    c=4,5 (right). Chunk c of block t holds kpos = (2t-2+c)*128 + p for
    query i = 256t + r, so kpos - i = (c-2)*128 + p - r and the band
    condition |kpos - i| <= 256 reduces to:
      c=0: p >= r      c=1: r <= p+128      c=4: p <= r      c=5: r >= p+128
    (kpos < G exclusion is handled by zeroed stationary rows, not masks.)
    """
    p = np.arange(128)[:, None]
    r = np.arange(256)[None, :]
    c0 = (p >= r)
    c1 = (r <= p + 128)
    c4 = (p <= r)
    c5 = (r >= p + 128)
    maskL = np.concatenate([c0, c1], axis=1).astype(np.float32)  # [128, 512]
    maskR = np.concatenate([c4, c5], axis=1).astype(np.float32)
    return np.stack([maskL, maskR], axis=1)  # [128, 2, 512]


def _patch_drain_and_barrier():
    """The walrus build in this container rejects >1 sync-wait on the CTRL
    (Drain) instruction that TileContext emits at exit ("Too many sync wait
    commands"). Split the waits: keep one on the drain, emit the rest as
    explicit single-sem wait_ge instructions on the sync engine before the
    barrier. Semantics preserved: all sems still quiesce before the
    sem-clear + barrier."""
    import concourse.tile as tile
    from concourse import mybir
    from concourse.vector_clock import ScopedClock

    if getattr(tile.TileContext, "_ant_drain_patch", False):
        return

    def _drain_and_barrier(self, tick_clock, wait_clock):
        nc = self.nc
        drain_inst = nc.sync.drain()
        wait_clock.add_sem_waits(
            drain_inst.ins, ScopedClock({None: tick_clock.global_clock})
        )
        si = drain_inst.ins.sync_info
        waits = list(si.on_wait) if si is not None else []
        if len(waits) > 1:
            drain_inst.ins.sync_info = mybir.SyncInfo(
                on_wait=[waits[0]], on_update=list(si.on_update)
            )
            allocated = self.sems.allocated()
            by_name = {}
            for key, sem in allocated.items():
                by_name[str(key)] = sem
                nm = getattr(sem, "name", None)
                if nm is not None:
                    by_name[str(nm)] = sem
            for w in waits[1:]:
                sem = by_name[w.ant_name]
                nc.sync.wait_ge(sem, w.wait_value)
        nc.all_engine_barrier()
        assert self.sems is not None
        popped = nc._tile_sem_poison_stack.pop()
        assert popped is self._sem_poison
        nc.clear_and_free_semaphores(list(self.sems.allocated().values()))
        nc.all_engine_barrier()

    tile.TileContext._drain_and_barrier = _drain_and_barrier
    tile.TileContext._ant_drain_patch = True


def _build_program():
    import concourse.bass as bass
    import concourse.tile as tile
    from concourse import bacc, mybir

    _patch_drain_and_barrier()

    f32 = mybir.dt.float32
    bf16 = mybir.dt.bfloat16
    AF = mybir.ActivationFunctionType
    AFexp = AF.Exp

    nc = bacc.Bacc(None)

    xT = nc.dram_tensor("xT", [Dm, S], bf16, kind="ExternalInput")
    Wqkkg = nc.dram_tensor("Wqkkg", [Dm, 3 * DPC], bf16, kind="ExternalInput")
    Wvvg = nc.dram_tensor("Wvvg", [Dm, 2 * DPC], bf16, kind="ExternalInput")
    Wqg = nc.dram_tensor("Wqg", [Dm, DPC], bf16, kind="ExternalInput")
    # bias5[r, m] = fused q/k/kg bias for output col 128*m + r
    bias5 = nc.dram_tensor("bias5", [128, 5], f32, kind="ExternalInput")
    b_qg = nc.dram_tensor("b_qg", [HD, HPC], f32, kind="ExternalInput")
    # broadcast v/vg bias: [128 partitions, head, (v|vg), 64]
    b_vvg = nc.dram_tensor("b_vvg", [128, HPC, 2, HD], f32, kind="ExternalInput")
    out_d = nc.dram_tensor("out", [DPC, S], f32, kind="ExternalOutput")

    masks_d = nc.inline_tensor(
        _build_masks().astype(ml_dtypes.bfloat16), name="masks"
    )
    eye16_d = nc.inline_tensor(np.eye(16, dtype=np.float32), name="eye16")

    from contextlib import ExitStack

    with tile.TileContext(nc) as tc, ExitStack() as ctx:
        const = ctx.enter_context(tc.tile_pool(name="const", bufs=1))
        ph = ctx.enter_context(tc.tile_pool(name="ph", bufs=1))
        xpool = ctx.enter_context(tc.tile_pool(name="xpool", bufs=2))
        bx = ctx.enter_context(tc.tile_pool(name="bx", bufs=6))
        pos_pool = ctx.enter_context(tc.tile_pool(name="posp", bufs=9))
        recs_pool = ctx.enter_context(tc.tile_pool(name="recsp", bufs=9))
        egt_pool = ctx.enter_context(tc.tile_pool(name="egtp", bufs=3))
        sbS = ctx.enter_context(tc.tile_pool(name="sbS", bufs=4))
        # PSUM (8 banks of 2KB/partition): psS 3 + psP 2 + psPO 2 + psG 1
        psS = ctx.enter_context(tc.tile_pool(name="psS", bufs=3, space="PSUM"))
        psP = ctx.enter_context(tc.tile_pool(name="psP", bufs=2, space="PSUM"))
        psPO = ctx.enter_context(tc.tile_pool(name="psPO", bufs=2, space="PSUM"))
        psG = ctx.enter_context(tc.tile_pool(name="psG", bufs=1, space="PSUM"))

        # ---- weights / constants to SBUF (first projection operands first) ----
        # startup-critical order: first M-chunk weights, then x-tile-0 in
        # kc-pair pieces interleaved with the remaining weight chunks, so the
        # first projection group starts ~1.4us in and never starves.
        w6 = const.tile([128, 6, 3 * DPC], bf16, tag="w6qkkg", name="w6qkkg")
        xt0 = xpool.tile([128, 6, 512], bf16, tag="xt", name="xt")
        nc.sync.dma_start(
            out=w6[:, :, 0:128],
            in_=Wqkkg[:, 0:128].rearrange("(c p) d -> p c d", p=128),
        )
        for kc2 in range(3):
            nc.sync.dma_start(
                out=xt0[:, 2 * kc2 : 2 * kc2 + 2, :],
                in_=xT[256 * kc2 : 256 * (kc2 + 1), 0:512].rearrange(
                    "(c p) s -> p c s", p=128
                ),
            )
        for m in range(1, 5):
            mw = 128 if m < 4 else 64
            nc.sync.dma_start(
                out=w6[:, :, 128 * m : 128 * m + mw],
                in_=Wqkkg[:, 128 * m : 128 * m + mw].rearrange(
                    "(c p) d -> p c d", p=128
                ),
            )
        w6vvg = const.tile([128, 6, 2 * DPC], bf16, tag="w6vvg", name="w6vvg")
        for kc2 in range(3):
            nc.sync.dma_start(
                out=w6vvg[:, 2 * kc2 : 2 * kc2 + 2, :],
                in_=Wvvg[256 * kc2 : 256 * (kc2 + 1), :].rearrange(
                    "(c p) d -> p c d", p=128
                ),
            )
        w6qg = const.tile([128, 6, DPC], bf16, tag="w6qg", name="w6qg")
        nc.sync.dma_start(
            out=w6qg, in_=Wqg[:, :].rearrange("(c p) d -> p c d", p=128)
        )
        bias5_sb = const.tile([128, 5], f32, tag="bias5", name="bias5_sb")
        nc.sync.dma_start(out=bias5_sb, in_=bias5[:])
        bqg_sb = const.tile([HD, HPC], f32, tag="bqg", name="bqg_sb")
        nc.sync.dma_start(out=bqg_sb, in_=b_qg[:])
        bvvg_sb = const.tile([128, HPC, 2, HD], f32, tag="bvvg", name="bvvg_sb")
        nc.sync.dma_start(out=bvvg_sb, in_=b_vvg[:])
        masks_sb = const.tile([128, 2, 512], bf16, tag="masks", name="masks_sb")
        nc.sync.dma_start(out=masks_sb, in_=masks_d[:])
        eye16_sb = const.tile([16, 16], f32, tag="eye16", name="eye16_sb")
        nc.sync.dma_start(out=eye16_sb, in_=eye16_d[:])

        # ---- persistent tensors ----
        qT = [ph.tile([64, S], bf16, tag=f"qT{h}", name=f"qT{h}") for h in range(HPC)]
        kT = [ph.tile([64, S], bf16, tag=f"kT{h}", name=f"kT{h}") for h in range(HPC)]
        kgT = [ph.tile([64, S], bf16, tag=f"kgT{h}", name=f"kgT{h}") for h in range(HPC)]
        # band PV stationary: [:, chunk, h, 0:64] = v head h, [:, :, :, 64:128] = 1.0
        vband = ph.tile([128, NKC, HPC, 128], bf16, tag="vband", name="vband")
        nc.gpsimd.memset(vband[:, :, :, HD:128], 1.0)
        # global PV moving operand: vg with a ones column (denominator)
        vgall = ph.tile([128, NKC, HPC, HD + 1], bf16, tag="vgall", name="vgall")
        nc.gpsimd.memset(vgall[:, :, :, HD : HD + 1], 1.0)
        # band-softmax copy of chunk-0 stationary rows 0..15 kept before
        # zeroing; head h at partitions 32h..32h+15 so the sel matmul's
        # operands share a base partition.
        vsel = ph.tile([80, 128], bf16, tag="vsel", name="vsel")
        nc.gpsimd.memset(vsel[:, HD:128], 1.0)
        # sel scores exp'd, head h at partitions 32h..32h+15
        selexp3 = ph.tile([96, S], bf16, tag="selexp3", name="selexp3")
        qgT = [ph.tile([64, G], bf16, tag=f"qgT{h}", name=f"qgT{h}") for h in range(HPC)]
        # natural-layout exp'd global scores [kpos, g] per head
        eg_nat = [
            ph.tile([128, NKC, G], bf16, tag=f"egn{h}", name=f"egn{h}")
            for h in range(HPC)
        ]
        outgS = [
            ph.tile([G, HD], f32, tag=f"outgS{h}", name=f"outgS{h}")
            for h in range(HPC)
        ]
        # block-0 output staging: written at st=1, global-row columns
        # overwritten at the tail, then DMA'd
        osb0 = ph.tile([64, HPC, 256], f32, tag="osb0", name="osb0")
        # global PV accumulator [16, 3*85] (head h at free offset 85h)
        po_g = psG.tile([G, 512], f32, tag="pog", name="po_g")

        def mm(out, lhsT, rhs, start, stop):
            nc.tensor.matmul(out, lhsT, rhs, start=start, stop=stop)

        # fused q/k/kg eviction map: M-chunk m rows [r0, r1) -> (dst, head)
        seg_map = {
            0: [(qT, 0, 0, 64), (qT, 1, 64, 128)],
            1: [(qT, 2, 0, 64), (kT, 0, 64, 128)],
            2: [(kT, 1, 0, 64), (kT, 2, 64, 128)],
            3: [(kgT, 0, 0, 64), (kgT, 1, 64, 128)],
            4: [(kgT, 2, 0, 64)],
        }

        # state for the pipelined band loop
        bexp_t = {}   # t -> [bexp tile per head]
        poS_t = {}    # t -> [(poS, recS) per head]

        def proj(st, xt):
            ssl = slice(512 * st, 512 * (st + 1))
            # q/k/kg fused: 5 M-chunks x 6 kc
            for m in range(5):
                mw = 128 if m < 4 else 64
                ps = psP.tile([128, 512], f32, tag="p512", name="psproj")
                for kc in range(6):
                    mm(
                        ps[0:mw, :],
                        w6[:, kc, 128 * m : 128 * m + mw],
                        xt[:, kc, :],
                        kc == 0,
                        kc == 5,
                    )
                for dst, h, r0, r1 in seg_map[m]:
                    if r0 == 0:
                        # same partitions: scalar engine (frees DVE)
                        nc.scalar.add(
                            dst[h][:, ssl],
                            ps[r0:r1, :],
                            bias5_sb[r0:r1, m : m + 1],
                        )
                    else:
                        # partition-shifted eviction: proven on DVE
                        nc.vector.tensor_scalar_add(
                            dst[h][:, ssl],
                            ps[r0:r1, :],
                            bias5_sb[r0:r1, m : m + 1],
                        )
            # v/vg: natural layout, xT chunks stationary
            for sc in range(4):
                ci = 4 * st + sc
                msl = slice(128 * sc, 128 * (sc + 1))
                psv = psP.tile([128, 512], f32, tag="p512", name="psv")
                for kc in range(6):
                    mm(
                        psv[:, 0 : 2 * DPC],
                        xt[:, kc, msl],
                        w6vvg[:, kc, :],
                        kc == 0,
                        kc == 5,
                    )
                # psv cols: g*192 + h*64 + d  (g = 0: v, 1: vg)
                src_v = bass.AP(
                    tensor=psv.tensor,
                    offset=psv.offset,
                    ap=[psv.ap[0], [HD, HPC], [1, HD]],
                )
                src_vg = bass.AP(
                    tensor=psv.tensor,
                    offset=psv.offset + DPC,
                    ap=[psv.ap[0], [HD, HPC], [1, HD]],
                )
                nc.vector.tensor_add(
                    vband[:, ci, :, 0:HD], src_v, bvvg_sb[:, :, 0, :]
                )
                nc.vector.tensor_add(
                    vgall[:, ci, :, 0:HD], src_vg, bvvg_sb[:, :, 1, :]
                )
                if ci == 0:
                    # v rows 0..15 for the sel PV (head h at partitions
                    # 32h..32h+15, matching selexp3), then zero the band
                    # copy of chunk-0 rows 0..15 (kpos < G exclusion)
                    for h in range(HPC):
                        src_h = bass.AP(
                            tensor=psv.tensor,
                            offset=psv.offset + HD * h,
                            ap=[[psv.ap[0][0], G], [1, HD]],
                        )
                        nc.vector.tensor_add(
                            vsel[32 * h : 32 * h + G, 0:HD],
                            src_h,
                            bvvg_sb[0:G, h, 0, :],
                        )
                    for h in range(HPC):
                        nc.gpsimd.memset(vband[0:G, 0, h, :], 0.0)
            # sel scores: col-tiled 3-head matmuls, one PSUM [80, 512]
            sps = psP.tile([128, 512], f32, tag="p512", name="sps")
            for h in range(HPC):
                mm(
                    sps[32 * h : 32 * h + G, :],
                    kT[h][:, 0:G],
                    qT[h][:, ssl],
                    True,
                    True,
                )
            nc.scalar.activation(
                out=selexp3[0:80, ssl], in_=sps[0:80, :], func=AFexp
            )
            if st == 0:
                # qg: [64, 16] per head, transposed
                for h in range(HPC):
                    psq = psP.tile([128, 512], f32, tag="p512", name="psqg")
                    for kc in range(6):
                        mm(
                            psq[0:64, 0:G],
                            w6qg[:, kc, HD * h : HD * (h + 1)],
                            xt[:, kc, 0:G],
                            kc == 0,
                            kc == 5,
                        )
                    nc.scalar.add(qgT[h], psq[0:64, 0:G], bqg_sb[:, h : h + 1])

        def glT(st):
            """Global-token scores for s-tile st, transposed [g, s]; then exp
            and xbar-transpose back to natural [kpos, g] layout."""
            ssl = slice(512 * st, 512 * (st + 1))
            gps = psP.tile([128, 512], f32, tag="p512", name="gps")
            for h in range(HPC):
                mm(
                    gps[32 * h : 32 * h + G, :],
                    qgT[h],
                    kgT[h][:, ssl],
                    True,
                    True,
                )
            egT = egt_pool.tile([96, 512], bf16, tag="egT", name="egT")
            nc.scalar.activation(out=egT[0:80, :], in_=gps[0:80, :], func=AFexp)
            for h in range(HPC):
                nc.sync.dma_start_transpose(
                    out=eg_nat[h][:, 4 * st : 4 * st + 4, :],
                    in_=egT[32 * h : 32 * h + G, :],
                )

        def ops_pv(st):
            """Global PV accumulation for s-tile st's 4 kpos chunks.

            All 96 matmuls form ONE accumulation group: start=True clears
            the whole bank's has_written bits, so per-head groups would
            lose their partials when a sibling head's group starts. A
            start=False write to a fresh element overwrites (has_written
            semantics), so one leading start + one trailing stop is exact.
            """
            for h in range(HPC):
                for cc in range(4):
                    c = 4 * st + cc
                    nc.tensor.matmul(
                        po_g[:, 85 * h : 85 * h + HD + 1],
                        eg_nat[h][:, c, :],
                        vgall[:, c, h, :],
                        start=(st == 0 and cc == 0 and h == 0),
                        stop=(st == NST - 1 and cc == 3 and h == HPC - 1),
                        skip_group_check=True,
                    )

        def band_qk(t):
            cl, ch = _chunk_range(t)
            qsl = slice(256 * t, 256 * (t + 1))
            tiles = []
            for h in range(HPC):
                bexp = bx.tile([128, 6, 256], bf16, tag="bexp", name="bexp")
                tiles.append(bexp)
                chunks = list(range(cl, ch))
                for s0 in range(0, len(chunks), 2):
                    sub = chunks[s0 : s0 + 2]
                    ps = psS.tile([128, 2, 256], f32, tag="qk", name="ps_qk")
                    for i, c in enumerate(sub):
                        j = 2 * t - 2 + c
                        mm(
                            ps[:, i, :],
                            kT[h][:, 128 * j : 128 * (j + 1)],
                            qT[h][:, qsl],
                            True,
                            True,
                        )
                    nc.scalar.activation(
                        out=bexp[:, sub[0] : sub[0] + len(sub), :],
                        in_=ps[:, 0 : len(sub), :],
                        func=AFexp,
                    )
                if t > 0:
                    nc.vector.tensor_mul(
                        bexp[:, 0:2, :].rearrange("p c q -> p (c q)"),
                        bexp[:, 0:2, :].rearrange("p c q -> p (c q)"),
                        masks_sb[:, 0, :],
                    )
                if t < NB - 1:
                    nc.vector.tensor_mul(
                        bexp[:, 4:6, :].rearrange("p c q -> p (c q)"),
                        bexp[:, 4:6, :].rearrange("p c q -> p (c q)"),
                        masks_sb[:, 1, :],
                    )
            bexp_t[t] = tiles

        def band_pv(t):
            cl, ch = _chunk_range(t)
            qsl = slice(256 * t, 256 * (t + 1))
            tiles = []
            po_tile = None
            for h in range(HPC):
                bexp = bexp_t[t][h]
                if h % 2 == 0:
                    po_tile = psPO.tile([128, 512], f32, tag="pv", name="po")
                po = po_tile[:, 256 * (h % 2) : 256 * (h % 2 + 1)]
                for i, c in enumerate(range(cl, ch)):
                    j = 2 * t - 2 + c
                    mm(po, vband[:, j, h, :], bexp[:, c, :], i == 0, False)
                mm(
                    po,
                    vsel[32 * h : 32 * h + G, :],
                    selexp3[32 * h : 32 * h + G, qsl],
                    False,
                    True,
                )
                # evict numerator (bf16) and reciprocal of the replicated
                # denominator rows; frees the PSUM bank quickly
                poS = pos_pool.tile([64, 256], bf16, tag="poS", name="poS")
                nc.scalar.copy(out=poS, in_=po[0:64, :])
                # exact reciprocal is an iterative-divide (~8 cyc/elem: ~2us
                # for [64, 256]); approx_fast is ~5x faster at 18 bits, far
                # below the bf16 noise floor. Denominators are positive and
                # O(10..600) so the undefined edge cases cannot occur. The
                # custom op needs an SBUF source at a matching base partition,
                # so copy-shift the replicated denominator rows out first.
                denS = recs_pool.tile([64, 256], f32, tag="denS", name="denS", bufs=3)
                nc.vector.tensor_copy(out=denS, in_=po[64:128, :])
                recS = recs_pool.tile([64, 256], f32, tag="recS", name="recS")
                nc.vector.reciprocal_approx_fast(out=recS, in_=denS)
                tiles.append((poS, recS))
            poS_t[t] = tiles
            del bexp_t[t]

        def band_postB(t):
            """Normalize and store block t."""
            eng = nc.gpsimd if (t <= 11) else nc.vector
            for h in range(HPC):
                poS, recS = poS_t[t][h]
                if t == 0:
                    eng.tensor_mul(osb0[:, h, :], poS, recS)
                else:
                    osbT = sbS.tile([64, 256], f32, tag="osbT", name="osbT")
                    eng.tensor_mul(osbT, poS, recS)
                    nc.sync.dma_start(
                        out=out_d[HD * h : HD * (h + 1), 256 * t : 256 * (t + 1)],
                        in_=osbT,
                    )
            del poS_t[t]

        def outg_chain():
            """Normalize global rows, transpose [16,64] -> [64,16], and
            overwrite cols 0..15 of block 0's staged output."""
            for h in range(HPC):
                recg = sbS.tile([G, 1], f32, tag="recg", name="recg")
                nc.vector.reciprocal(
                    recg, po_g[:, 85 * h + HD : 85 * h + HD + 1]
                )
                nc.vector.tensor_scalar_mul(
                    outgS[h], po_g[:, 85 * h : 85 * h + HD], recg
                )
            for h in range(HPC):
                trsp = psPO.tile([128, 512], f32, tag="pv", name="trsp")
                nc.tensor.transpose(trsp[0:HD, 0:G], outgS[h], eye16_sb)
                nc.vector.tensor_copy(out=osb0[:, h, 0:G], in_=trsp[0:HD, 0:G])
                nc.sync.dma_start(
                    out=out_d[HD * h : HD * (h + 1), 0:256], in_=osb0[:, h, :]
                )

        # ================= emission schedule =================
        # Band block t's QK needs kpos chunks <= 2t+3, available after
        # s-tile st = ceil((2t+3)/4) - ... i.e. blocks 2st-1, 2st issue at
        # s-tile st. PV trails QK by ~one block (hides exp+masks), postB
        # trails PV by ~two (hides the DVE normalize chain). PV(t) is
        # emitted before QK(t+2) so bexp pool recycling never outruns its
        # readers. Block 0 runs at st=1 (deps are ready); only its
        # global-row overwrite and DMA stay at the tail.
        proj(0, xt0)

        for st in range(1, NST):
            xt = xpool.tile([128, 6, 512], bf16, tag="xt", name="xt")
            nc.sync.dma_start(
                out=xt,
                in_=xT[:, 512 * st : 512 * (st + 1)].rearrange(
                    "(c p) s -> p c s", p=128
                ),
            )
            glT(st - 1)
            proj(st, xt)
            if st == 1:
                band_qk(1)
                band_qk(0)
                ops_pv(0)
                band_pv(0)
                band_qk(2)
            else:
                band_pv(2 * st - 3)
                band_qk(2 * st - 1)
                ops_pv(st - 1)
                if st == 2:
                    band_postB(0)
                if 2 * st - 5 >= 1:
                    band_postB(2 * st - 5)
                band_pv(2 * st - 2)
                band_qk(2 * st)
                if 2 * st - 4 >= 1:
                    band_postB(2 * st - 4)
                if st == NST - 1:
                    band_pv(2 * st - 1)
                    band_qk(2 * st + 1)
                    band_postB(2 * st - 3)
                    glT(st)
                    band_pv(2 * st)
                    ops_pv(st)

        # tail: remaining band blocks and the global rows
        outg_chain()
        band_postB(NB - 4)
        band_pv(NB - 1)
        band_postB(NB - 3)
        band_postB(NB - 2)
        band_postB(NB - 1)

    return nc


def _get_program():
    if "nc" not in _CACHE:
        nc = _build_program()
        nc.finalize()
        _CACHE["nc"] = nc
    return _CACHE["nc"]


def _prep_in_maps(hidden_states, Wq, bq, Wk, bk, Wv, bv, Wqg, bqg, Wkg, bkg, Wvg, bvg):
    hs = np.asarray(hidden_states, dtype=np.float32)
    f32 = np.float32
    bfl = ml_dtypes.bfloat16
    in_maps = []
    for c in range(NCORES):
        b = c // 4
        cols = slice(HD * 3 * (c % 4), HD * (3 * (c % 4) + 3))

        Wqkkg = np.concatenate(
            [
                np.asarray(Wq)[:, cols] * SCALE,
                np.asarray(Wk)[:, cols],
                np.asarray(Wkg)[:, cols],
            ],
            axis=1,
        )  # [768, 576]
        bflat = np.concatenate(
            [
                np.asarray(bq)[cols] * SCALE,
                np.asarray(bk)[cols],
                np.asarray(bkg)[cols],
            ]
        )  # [576]
        bias5 = np.zeros((128, 5), f32)
        for m in range(5):
            mw = 128 if m < 4 else 64
            bias5[0:mw, m] = bflat[128 * m : 128 * m + mw]

        bvvg = np.stack(
            [
                np.asarray(bv)[cols].reshape(HPC, HD),
                np.asarray(bvg)[cols].reshape(HPC, HD),
            ],
            axis=1,
        ).astype(f32)  # [3, 2, 64]
        bqg_col = np.ascontiguousarray(
            (np.asarray(bqg)[cols] * SCALE).reshape(HPC, HD).T.astype(f32)
        )
        in_maps.append(
            {
                "xT": np.ascontiguousarray(hs[b].T).astype(bfl),
                "Wqkkg": np.ascontiguousarray(Wqkkg).astype(bfl),
                "Wvvg": np.concatenate(
                    [np.asarray(Wv)[:, cols], np.asarray(Wvg)[:, cols]], axis=1
                ).astype(bfl),
                "Wqg": np.ascontiguousarray(np.asarray(Wqg)[:, cols] * SCALE).astype(
                    bfl
                ),
                "bias5": bias5,
                "b_qg": bqg_col,
                "b_vvg": np.ascontiguousarray(
                    np.broadcast_to(bvvg[None], (128, HPC, 2, HD))
                ),
            }
        )
    return in_maps


def kernel(
    hidden_states,
    Wq,
    bq,
    Wk,
    bk,
    Wv,
    bv,
    Wqg,
    bqg,
    Wkg,
    bkg,
    Wvg,
    bvg,
    n_global,
):
    from concourse.bass_utils import run_bass_kernel_spmd

    assert int(n_global) == G
    nc = _get_program()
    in_maps = _prep_in_maps(
        hidden_states, Wq, bq, Wk, bk, Wv, bv, Wqg, bqg, Wkg, bkg, Wvg, bvg
    )
    res = run_bass_kernel_spmd(nc, in_maps, list(range(NCORES)))
    out = np.zeros((B, S, Dm), np.float32)
    for c in range(NCORES):
        b = c // 4
        cols = slice(HD * 3 * (c % 4), HD * (3 * (c % 4) + 3))
        out[b, :, cols] = res.results[c]["out"].T
    return out


# revision 23
# speedup vs baseline: 1.1790x; 1.1790x over previous
"""Longformer self-attention Trainium2 kernel (8-core SPMD), v2.

Sharding: core c handles batch b = c//4 and heads [3*(c%4), 3*(c%4)+3).
Each core computes its 3 heads' output transposed [192, 4096]; the host
reassembles [2, 4096, 768] (transposing back).

v2 structural changes vs the v1 baseline (285us):
  - Band PV computed transposed: out[d, q] = sum_p v[p, d] * bexp[p, q]
    with a [128, 128] stationary (v head cols 0..63 | 64 ones columns):
    the moving dim rises to 256 and the softmax denominator comes out
    REPLICATED on PSUM partitions 64..127 (each ones column sums bexp),
    so normalization is reciprocal + one tensor_tensor mul -- no
    partition broadcast needed. This replaces v1's 14 LDWEIGHTS-bound
    N=65 matmuls per (block, head) with 7 N=256 matmuls.
  - q/k/kg projections fused into one [768, 576] weight so all matmuls
    except one are full M=128; same-partition bias evictions move to
    the scalar engine (frees DVE), partition-shifted ones stay on DVE.
  - Global-token scores computed transposed via col-tiled (3 heads
    concurrent) N=512 matmuls, then moved to natural [kpos, g] layout
    with the DMA xbar transpose engine; global PV keeps the cheap
    16-col-stationary accumulation.
  - kpos<G exclusion from the band softmax implemented by zeroing
    rows 0..15 of the band stationary for chunk 0 (v AND ones columns):
    numerator and denominator contributions vanish exactly, matching
    exp(-10000 + x) == 0 in f32. The remaining band-validity masks are
    two paired [128, 512] multiplies per block.
  - Band blocks are interleaved into the projection phase (block t
    issues once kpos chunk 2t+3 exists) so the PE stream stays dense:
    the v1 trace showed the HAM clock-gate dropping to K=4/8 at ~142us
    and never recovering -- half the kernel ran at 1.2 GHz.
"""

import sys

sys.path.insert(0, "/opt/trn_rl_repo")

import numpy as np
import ml_dtypes

B, S, Dm, H, WIN, G, HD = 2, 4096, 768, 12, 256, 16, 64
HPC = 3            # heads per core
NCORES = 8
DPC = HPC * HD     # 192 output dims per core
NB = S // WIN      # 16 query blocks
NKC = S // 128     # 32 kpos chunks of 128
NST = 8            # 512-wide s-tiles
SCALE = 1.0 / 8.0  # 1/sqrt(64)

_CACHE = {}


def _chunk_range(t):
    if t == 0:
        return 2, 6
    if t == NB - 1:
        return 0, 4
    return 0, 6


def _build_masks():
    """Validity masks in transposed-score orientation [kpos_local p, q r].

    maskL covers window chunks c=0,1 (left boundary), maskR covers
    c=4,5 (right). Chunk c of block t holds kpos = (2t-2+c)*128 + p for
    query i = 256t + r, so kpos - i = (c-2)*128 + p - r and the band
    condition |kpos - i| <= 256 reduces to:
      c=0: p >= r      c=1: r <= p+128      c=4: p <= r      c=5: r >= p+128
    (kpos < G exclusion is handled by zeroed stationary rows, not masks.)
    """
    p = np.arange(128)[:, None]
    r = np.arange(256)[None, :]
    c0 = (p >= r)
    c1 = (r <= p + 128)
    c4 = (p <= r)
    c5 = (r >= p + 128)
    maskL = np.concatenate([c0, c1], axis=1).astype(np.float32)  # [128, 512]
    maskR = np.concatenate([c4, c5], axis=1).astype(np.float32)
    return np.stack([maskL, maskR], axis=1)  # [128, 2, 512]


def _patch_drain_and_barrier():
    """The walrus build in this container rejects >1 sync-wait on the CTRL
    (Drain) instruction that TileContext emits at exit ("Too many sync wait
    commands"). Split the waits: keep one on the drain, emit the rest as
    explicit single-sem wait_ge instructions on the sync engine before the
    barrier. Semantics preserved: all sems still quiesce before the
    sem-clear + barrier."""
    import concourse.tile as tile
    from concourse import mybir
    from concourse.vector_clock import ScopedClock

    if getattr(tile.TileContext, "_ant_drain_patch", False):
        return

    def _drain_and_barrier(self, tick_clock, wait_clock):
        nc = self.nc
        drain_inst = nc.sync.drain()
        wait_clock.add_sem_waits(
            drain_inst.ins, ScopedClock({None: tick_clock.global_clock})
        )
        si = drain_inst.ins.sync_info
        waits = list(si.on_wait) if si is not None else []
        if len(waits) > 1:
            drain_inst.ins.sync_info = mybir.SyncInfo(
                on_wait=[waits[0]], on_update=list(si.on_update)
            )
            allocated = self.sems.allocated()
            by_name = {}
            for key, sem in allocated.items():
                by_name[str(key)] = sem
                nm = getattr(sem, "name", None)
                if nm is not None:
                    by_name[str(nm)] = sem
            for w in waits[1:]:
                sem = by_name[w.ant_name]
                nc.sync.wait_ge(sem, w.wait_value)
        nc.all_engine_barrier()
        assert self.sems is not None
        popped = nc._tile_sem_poison_stack.pop()
        assert popped is self._sem_poison
        nc.clear_and_free_semaphores(list(self.sems.allocated().values()))
        nc.all_engine_barrier()

    tile.TileContext._drain_and_barrier = _drain_and_barrier
    tile.TileContext._ant_drain_patch = True


def _build_program():
    import concourse.bass as bass
    import concourse.tile as tile
    from concourse import bacc, mybir

    _patch_drain_and_barrier()

    f32 = mybir.dt.float32
    bf16 = mybir.dt.bfloat16
    AF = mybir.ActivationFunctionType
    AFexp = AF.Exp

    nc = bacc.Bacc(None)

    xT = nc.dram_tensor("xT", [Dm, S], bf16, kind="ExternalInput")
    Wqkkg = nc.dram_tensor("Wqkkg", [Dm, 3 * DPC], bf16, kind="ExternalInput")
    Wvvg = nc.dram_tensor("Wvvg", [Dm, 2 * DPC], bf16, kind="ExternalInput")
    Wqg = nc.dram_tensor("Wqg", [Dm, DPC], bf16, kind="ExternalInput")
    # bias5[r, m] = fused q/k/kg bias for output col 128*m + r
    bias5 = nc.dram_tensor("bias5", [128, 5], f32, kind="ExternalInput")
    b_qg = nc.dram_tensor("b_qg", [HD, HPC], f32, kind="ExternalInput")
    # broadcast v/vg bias: [128 partitions, head, (v|vg), 64]
    b_vvg = nc.dram_tensor("b_vvg", [128, HPC, 2, HD], f32, kind="ExternalInput")
    out_d = nc.dram_tensor("out", [DPC, S], f32, kind="ExternalOutput")

    masks_d = nc.inline_tensor(
        _build_masks().astype(ml_dtypes.bfloat16), name="masks"
    )
    eye16_d = nc.inline_tensor(np.eye(16, dtype=np.float32), name="eye16")

    from contextlib import ExitStack

    with tile.TileContext(nc) as tc, ExitStack() as ctx:
        const = ctx.enter_context(tc.tile_pool(name="const", bufs=1))
        ph = ctx.enter_context(tc.tile_pool(name="ph", bufs=1))
        xpool = ctx.enter_context(tc.tile_pool(name="xpool", bufs=2))
        bx = ctx.enter_context(tc.tile_pool(name="bx", bufs=6))
        pos_pool = ctx.enter_context(tc.tile_pool(name="posp", bufs=9))
        recs_pool = ctx.enter_context(tc.tile_pool(name="recsp", bufs=9))
        egt_pool = ctx.enter_context(tc.tile_pool(name="egtp", bufs=3))
        sbS = ctx.enter_context(tc.tile_pool(name="sbS", bufs=4))
        # PSUM (8 banks of 2KB/partition): psS 3 + psP 2 + psPO 2 + psG 1
        psS = ctx.enter_context(tc.tile_pool(name="psS", bufs=3, space="PSUM"))
        psP = ctx.enter_context(tc.tile_pool(name="psP", bufs=2, space="PSUM"))
        psPO = ctx.enter_context(tc.tile_pool(name="psPO", bufs=2, space="PSUM"))
        psG = ctx.enter_context(tc.tile_pool(name="psG", bufs=1, space="PSUM"))

        # ---- weights / constants to SBUF (first projection operands first) ----
        # startup-critical order: first M-chunk weights, then x-tile-0 in
        # kc-pair pieces interleaved with the remaining weight chunks, so the
        # first projection group starts ~1.4us in and never starves.
        w6 = const.tile([128, 6, 3 * DPC], bf16, tag="w6qkkg", name="w6qkkg")
        xt0 = xpool.tile([128, 6, 512], bf16, tag="xt", name="xt")
        nc.sync.dma_start(
            out=w6[:, :, 0:128],
            in_=Wqkkg[:, 0:128].rearrange("(c p) d -> p c d", p=128),
        )
        for kc2 in range(3):
            nc.sync.dma_start(
                out=xt0[:, 2 * kc2 : 2 * kc2 + 2, :],
                in_=xT[256 * kc2 : 256 * (kc2 + 1), 0:512].rearrange(
                    "(c p) s -> p c s", p=128
                ),
            )
        for m in range(1, 5):
            mw = 128 if m < 4 else 64
            nc.sync.dma_start(
                out=w6[:, :, 128 * m : 128 * m + mw],
                in_=Wqkkg[:, 128 * m : 128 * m + mw].rearrange(
                    "(c p) d -> p c d", p=128
                ),
            )
        w6vvg = const.tile([128, 6, 2 * DPC], bf16, tag="w6vvg", name="w6vvg")
        for kc2 in range(3):
            nc.sync.dma_start(
                out=w6vvg[:, 2 * kc2 : 2 * kc2 + 2, :],
                in_=Wvvg[256 * kc2 : 256 * (kc2 + 1), :].rearrange(
                    "(c p) d -> p c d", p=128
                ),
            )
        w6qg = const.tile([128, 6, DPC], bf16, tag="w6qg", name="w6qg")
        nc.sync.dma_start(
            out=w6qg, in_=Wqg[:, :].rearrange("(c p) d -> p c d", p=128)
        )
        bias5_sb = const.tile([128, 5], f32, tag="bias5", name="bias5_sb")
        nc.sync.dma_start(out=bias5_sb, in_=bias5[:])
        bqg_sb = const.tile([HD, HPC], f32, tag="bqg", name="bqg_sb")
        nc.sync.dma_start(out=bqg_sb, in_=b_qg[:])
        bvvg_sb = const.tile([128, HPC, 2, HD], f32, tag="bvvg", name="bvvg_sb")
        nc.sync.dma_start(out=bvvg_sb, in_=b_vvg[:])
        masks_sb = const.tile([128, 2, 512], bf16, tag="masks", name="masks_sb")
        nc.sync.dma_start(out=masks_sb, in_=masks_d[:])
        eye16_sb = const.tile([16, 16], f32, tag="eye16", name="eye16_sb")
        nc.sync.dma_start(out=eye16_sb, in_=eye16_d[:])

        # ---- persistent tensors ----
        qT = [ph.tile([64, S], bf16, tag=f"qT{h}", name=f"qT{h}") for h in range(HPC)]
        kT = [ph.tile([64, S], bf16, tag=f"kT{h}", name=f"kT{h}") for h in range(HPC)]
        kgT = [ph.tile([64, S], bf16, tag=f"kgT{h}", name=f"kgT{h}") for h in range(HPC)]
        # band PV stationary: [:, chunk, h, 0:64] = v head h, [:, :, :, 64:128] = 1.0
        vband = ph.tile([128, NKC, HPC, 128], bf16, tag="vband", name="vband")
        nc.vector.memset(vband[:, :, :, HD:128], 1.0)
        # global PV moving operand: vg with a ones column (denominator)
        vgall = ph.tile([128, NKC, HPC, HD + 1], bf16, tag="vgall", name="vgall")
        nc.vector.memset(vgall[:, :, :, HD : HD + 1], 1.0)
        # band-softmax copy of chunk-0 stationary rows 0..15 kept before
        # zeroing; head h at partitions 32h..32h+15 so the sel matmul's
        # operands share a base partition.
        vsel = ph.tile([80, 128], bf16, tag="vsel", name="vsel")
        nc.vector.memset(vsel[:, HD:128], 1.0)
        # sel scores exp'd, head h at partitions 32h..32h+15
        selexp3 = ph.tile([96, S], bf16, tag="selexp3", name="selexp3")
        qgT = [ph.tile([64, G], bf16, tag=f"qgT{h}", name=f"qgT{h}") for h in range(HPC)]
        # natural-layout exp'd global scores [kpos, g] per head
        eg_nat = [
            ph.tile([128, NKC, G], bf16, tag=f"egn{h}", name=f"egn{h}")
            for h in range(HPC)
        ]
        outgS = [
            ph.tile([G, HD], f32, tag=f"outgS{h}", name=f"outgS{h}")
            for h in range(HPC)
        ]
        # block-0 output staging: written at st=1, global-row columns
        # overwritten at the tail, then DMA'd
        osb0 = ph.tile([64, HPC, 256], f32, tag="osb0", name="osb0")
        # global PV accumulator [16, 3*85] (head h at free offset 85h)
        po_g = psG.tile([G, 512], f32, tag="pog", name="po_g")

        def mm(out, lhsT, rhs, start, stop):
            nc.tensor.matmul(out, lhsT, rhs, start=start, stop=stop)

        # fused q/k/kg eviction map: M-chunk m rows [r0, r1) -> (dst, head)
        seg_map = {
            0: [(qT, 0, 0, 64), (qT, 1, 64, 128)],
            1: [(qT, 2, 0, 64), (kT, 0, 64, 128)],
            2: [(kT, 1, 0, 64), (kT, 2, 64, 128)],
            3: [(kgT, 0, 0, 64), (kgT, 1, 64, 128)],
            4: [(kgT, 2, 0, 64)],
        }

        # state for the pipelined band loop
        bexp_t = {}   # t -> [bexp tile per head]
        poS_t = {}    # t -> [(poS, recS) per head]

        def proj(st, xt):
            ssl = slice(512 * st, 512 * (st + 1))
            # q/k/kg fused: 5 M-chunks x 6 kc
            for m in range(5):
                mw = 128 if m < 4 else 64
                ps = psP.tile([128, 512], f32, tag="p512", name="psproj")
                for kc in range(6):
                    mm(
                        ps[0:mw, :],
                        w6[:, kc, 128 * m : 128 * m + mw],
                        xt[:, kc, :],
                        kc == 0,
                        kc == 5,
                    )
                for dst, h, r0, r1 in seg_map[m]:
                    if r0 == 0:
                        # same partitions: scalar engine (frees DVE)
                        nc.scalar.add(
                            dst[h][:, ssl],
                            ps[r0:r1, :],
                            bias5_sb[r0:r1, m : m + 1],
                        )
                    else:
                        # partition-shifted eviction: proven on DVE
                        nc.vector.tensor_scalar_add(
                            dst[h][:, ssl],
                            ps[r0:r1, :],
                            bias5_sb[r0:r1, m : m + 1],
                        )
            # v/vg: natural layout, xT chunks stationary
            for sc in range(4):
                ci = 4 * st + sc
                msl = slice(128 * sc, 128 * (sc + 1))
                psv = psP.tile([128, 512], f32, tag="p512", name="psv")
                for kc in range(6):
                    mm(
                        psv[:, 0 : 2 * DPC],
                        xt[:, kc, msl],
                        w6vvg[:, kc, :],
                        kc == 0,
                        kc == 5,
                    )
                # psv cols: g*192 + h*64 + d  (g = 0: v, 1: vg)
                src_v = bass.AP(
                    tensor=psv.tensor,
                    offset=psv.offset,
                    ap=[psv.ap[0], [HD, HPC], [1, HD]],
                )
                src_vg = bass.AP(
                    tensor=psv.tensor,
                    offset=psv.offset + DPC,
                    ap=[psv.ap[0], [HD, HPC], [1, HD]],
                )
                nc.vector.tensor_add(
                    vband[:, ci, :, 0:HD], src_v, bvvg_sb[:, :, 0, :]
                )
                nc.vector.tensor_add(
                    vgall[:, ci, :, 0:HD], src_vg, bvvg_sb[:, :, 1, :]
                )
                if ci == 0:
                    # v rows 0..15 for the sel PV (head h at partitions
                    # 32h..32h+15, matching selexp3), then zero the band
                    # copy of chunk-0 rows 0..15 (kpos < G exclusion)
                    for h in range(HPC):
                        src_h = bass.AP(
                            tensor=psv.tensor,
                            offset=psv.offset + HD * h,
                            ap=[[psv.ap[0][0], G], [1, HD]],
                        )
                        nc.vector.tensor_add(
                            vsel[32 * h : 32 * h + G, 0:HD],
                            src_h,
                            bvvg_sb[0:G, h, 0, :],
                        )
                    for h in range(HPC):
                        nc.vector.memset(vband[0:G, 0, h, :], 0.0)
            # sel scores: col-tiled 3-head matmuls, one PSUM [80, 512]
            sps = psP.tile([128, 512], f32, tag="p512", name="sps")
            for h in range(HPC):
                mm(
                    sps[32 * h : 32 * h + G, :],
                    kT[h][:, 0:G],
                    qT[h][:, ssl],
                    True,
                    True,
                )
            nc.scalar.activation(
                out=selexp3[0:80, ssl], in_=sps[0:80, :], func=AFexp
            )
            if st == 0:
                # qg: [64, 16] per head, transposed
                for h in range(HPC):
                    psq = psP.tile([128, 512], f32, tag="p512", name="psqg")
                    for kc in range(6):
                        mm(
                            psq[0:64, 0:G],
                            w6qg[:, kc, HD * h : HD * (h + 1)],
                            xt[:, kc, 0:G],
                            kc == 0,
                            kc == 5,
                        )
                    nc.scalar.add(qgT[h], psq[0:64, 0:G], bqg_sb[:, h : h + 1])

        def glT(st):
            """Global-token scores for s-tile st, transposed [g, s]; then exp
            and xbar-transpose back to natural [kpos, g] layout."""
            ssl = slice(512 * st, 512 * (st + 1))
            gps = psP.tile([128, 512], f32, tag="p512", name="gps")
            for h in range(HPC):
                mm(
                    gps[32 * h : 32 * h + G, :],
                    qgT[h],
                    kgT[h][:, ssl],
                    True,
                    True,
                )
            egT = egt_pool.tile([96, 512], bf16, tag="egT", name="egT")
            nc.scalar.activation(out=egT[0:80, :], in_=gps[0:80, :], func=AFexp)
            for h in range(HPC):
                nc.sync.dma_start_transpose(
                    out=eg_nat[h][:, 4 * st : 4 * st + 4, :],
                    in_=egT[32 * h : 32 * h + G, :],
                )

        def ops_pv(st):
            """Global PV accumulation for s-tile st's 4 kpos chunks.

            All 96 matmuls form ONE accumulation group: start=True clears
            the whole bank's has_written bits, so per-head groups would
            lose their partials when a sibling head's group starts. A
            start=False write to a fresh element overwrites (has_written
            semantics), so one leading start + one trailing stop is exact.
            """
            for h in range(HPC):
                for cc in range(4):
                    c = 4 * st + cc
                    nc.tensor.matmul(
                        po_g[:, 85 * h : 85 * h + HD + 1],
                        eg_nat[h][:, c, :],
                        vgall[:, c, h, :],
                        start=(st == 0 and cc == 0 and h == 0),
                        stop=(st == NST - 1 and cc == 3 and h == HPC - 1),
                        skip_group_check=True,
                    )

        def band_qk(t):
            cl, ch = _chunk_range(t)
            qsl = slice(256 * t, 256 * (t + 1))
            tiles = []
            for h in range(HPC):
                bexp = bx.tile([128, 6, 256], bf16, tag="bexp", name="bexp")
                tiles.append(bexp)
                chunks = list(range(cl, ch))
                for s0 in range(0, len(chunks), 2):
                    sub = chunks[s0 : s0 + 2]
                    ps = psS.tile([128, 2, 256], f32, tag="qk", name="ps_qk")
                    for i, c in enumerate(sub):
                        j = 2 * t - 2 + c
                        mm(
                            ps[:, i, :],
                            kT[h][:, 128 * j : 128 * (j + 1)],
                            qT[h][:, qsl],
                            True,
                            True,
                        )
                    nc.scalar.activation(
                        out=bexp[:, sub[0] : sub[0] + len(sub), :],
                        in_=ps[:, 0 : len(sub), :],
                        func=AFexp,
                    )
                if t > 0:
                    nc.vector.tensor_mul(
                        bexp[:, 0:2, :].rearrange("p c q -> p (c q)"),
                        bexp[:, 0:2, :].rearrange("p c q -> p (c q)"),
                        masks_sb[:, 0, :],
                    )
                if t < NB - 1:
                    nc.vector.tensor_mul(
                        bexp[:, 4:6, :].rearrange("p c q -> p (c q)"),
                        bexp[:, 4:6, :].rearrange("p c q -> p (c q)"),
                        masks_sb[:, 1, :],
                    )
            bexp_t[t] = tiles

        def band_pv(t):
            cl, ch = _chunk_range(t)
            qsl = slice(256 * t, 256 * (t + 1))
            tiles = []
            po_tile = None
            for h in range(HPC):
                bexp = bexp_t[t][h]
                if h % 2 == 0:
                    po_tile = psPO.tile([128, 512], f32, tag="pv", name="po")
                po = po_tile[:, 256 * (h % 2) : 256 * (h % 2 + 1)]
                for i, c in enumerate(range(cl, ch)):
                    j = 2 * t - 2 + c
                    mm(po, vband[:, j, h, :], bexp[:, c, :], i == 0, False)
                mm(
                    po,
                    vsel[32 * h : 32 * h + G, :],
                    selexp3[32 * h : 32 * h + G, qsl],
                    False,
                    True,
                )
                # evict numerator (bf16) and reciprocal of the replicated
                # denominator rows; frees the PSUM bank quickly
                poS = pos_pool.tile([64, 256], bf16, tag="poS", name="poS")
                nc.scalar.copy(out=poS, in_=po[0:64, :])
                # exact reciprocal is an iterative-divide (~8 cyc/elem: ~2us
                # for [64, 256]); approx_fast is ~5x faster at 18 bits, far
                # below the bf16 noise floor. Denominators are positive and
                # O(10..600) so the undefined edge cases cannot occur. The
                # custom op needs an SBUF source at a matching base partition,
                # so copy-shift the replicated denominator rows out first.
                denS = recs_pool.tile([64, 256], f32, tag="denS", name="denS", bufs=3)
                nc.vector.tensor_copy(out=denS, in_=po[64:128, :])
                recS = recs_pool.tile([64, 256], f32, tag="recS", name="recS")
                nc.vector.reciprocal_approx_fast(out=recS, in_=denS)
                tiles.append((poS, recS))
            poS_t[t] = tiles
            del bexp_t[t]

        def band_postB(t):
            """Normalize and store block t."""
            eng = nc.gpsimd if (t <= 11) else nc.vector
            for h in range(HPC):
                poS, recS = poS_t[t][h]
                if t == 0:
                    eng.tensor_mul(osb0[:, h, :], poS, recS)
                else:
                    osbT = sbS.tile([64, 256], f32, tag="osbT", name="osbT")
                    eng.tensor_mul(osbT, poS, recS)
                    nc.sync.dma_start(
                        out=out_d[HD * h : HD * (h + 1), 256 * t : 256 * (t + 1)],
                        in_=osbT,
                    )
            del poS_t[t]

        def outg_chain():
            """Normalize global rows, transpose [16,64] -> [64,16], and
            overwrite cols 0..15 of block 0's staged output."""
            for h in range(HPC):
                recg = sbS.tile([G, 1], f32, tag="recg", name="recg")
                nc.vector.reciprocal(
                    recg, po_g[:, 85 * h + HD : 85 * h + HD + 1]
                )
                nc.vector.tensor_scalar_mul(
                    outgS[h], po_g[:, 85 * h : 85 * h + HD], recg
                )
            for h in range(HPC):
                trsp = psPO.tile([128, 512], f32, tag="pv", name="trsp")
                nc.tensor.transpose(trsp[0:HD, 0:G], outgS[h], eye16_sb)
                nc.vector.tensor_copy(out=osb0[:, h, 0:G], in_=trsp[0:HD, 0:G])
                nc.sync.dma_start(
                    out=out_d[HD * h : HD * (h + 1), 0:256], in_=osb0[:, h, :]
                )

        # ================= emission schedule =================
        # Band block t's QK needs kpos chunks <= 2t+3, available after
        # s-tile st = ceil((2t+3)/4) - ... i.e. blocks 2st-1, 2st issue at
        # s-tile st. PV trails QK by ~one block (hides exp+masks), postB
        # trails PV by ~two (hides the DVE normalize chain). PV(t) is
        # emitted before QK(t+2) so bexp pool recycling never outruns its
        # readers. Block 0 runs at st=1 (deps are ready); only its
        # global-row overwrite and DMA stay at the tail.
        proj(0, xt0)

        for st in range(1, NST):
            xt = xpool.tile([128, 6, 512], bf16, tag="xt", name="xt")
            nc.sync.dma_start(
                out=xt,
                in_=xT[:, 512 * st : 512 * (st + 1)].rearrange(
                    "(c p) s -> p c s", p=128
                ),
            )
            glT(st - 1)
            proj(st, xt)
            if st == 1:
                band_qk(1)
                band_qk(0)
                ops_pv(0)
                band_pv(0)
                band_qk(2)
            else:
                band_pv(2 * st - 3)
                band_qk(2 * st - 1)
                ops_pv(st - 1)
                if st == 2:
                    band_postB(0)
                if 2 * st - 5 >= 1:
                    band_postB(2 * st - 5)
                band_pv(2 * st - 2)
                band_qk(2 * st)
                if 2 * st - 4 >= 1:
                    band_postB(2 * st - 4)
                if st == NST - 1:
                    band_pv(2 * st - 1)
                    band_qk(2 * st + 1)
                    band_postB(2 * st - 3)
                    glT(st)
                    band_pv(2 * st)
                    ops_pv(st)

        # tail: remaining band blocks and the global rows
        outg_chain()
        band_postB(NB - 4)
        band_pv(NB - 1)
        band_postB(NB - 3)
        band_postB(NB - 2)
        band_postB(NB - 1)

    return nc


def _get_program():
    if "nc" not in _CACHE:
        nc = _build_program()
        nc.finalize()
        _CACHE["nc"] = nc
    return _CACHE["nc"]


def _prep_in_maps(hidden_states, Wq, bq, Wk, bk, Wv, bv, Wqg, bqg, Wkg, bkg, Wvg, bvg):
    hs = np.asarray(hidden_states, dtype=np.float32)
    f32 = np.float32
    bfl = ml_dtypes.bfloat16
    in_maps = []
    for c in range(NCORES):
        b = c // 4
        cols = slice(HD * 3 * (c % 4), HD * (3 * (c % 4) + 3))

        Wqkkg = np.concatenate(
            [
                np.asarray(Wq)[:, cols] * SCALE,
                np.asarray(Wk)[:, cols],
                np.asarray(Wkg)[:, cols],
            ],
            axis=1,
        )  # [768, 576]
        bflat = np.concatenate(
            [
                np.asarray(bq)[cols] * SCALE,
                np.asarray(bk)[cols],
                np.asarray(bkg)[cols],
            ]
        )  # [576]
        bias5 = np.zeros((128, 5), f32)
        for m in range(5):
            mw = 128 if m < 4 else 64
            bias5[0:mw, m] = bflat[128 * m : 128 * m + mw]

        bvvg = np.stack(
            [
                np.asarray(bv)[cols].reshape(HPC, HD),
                np.asarray(bvg)[cols].reshape(HPC, HD),
            ],
            axis=1,
        ).astype(f32)  # [3, 2, 64]
        bqg_col = np.ascontiguousarray(
            (np.asarray(bqg)[cols] * SCALE).reshape(HPC, HD).T.astype(f32)
        )
        in_maps.append(
            {
                "xT": np.ascontiguousarray(hs[b].T).astype(bfl),
                "Wqkkg": np.ascontiguousarray(Wqkkg).astype(bfl),
                "Wvvg": np.concatenate(
                    [np.asarray(Wv)[:, cols], np.asarray(Wvg)[:, cols]], axis=1
                ).astype(bfl),
                "Wqg": np.ascontiguousarray(np.asarray(Wqg)[:, cols] * SCALE).astype(
                    bfl
                ),
                "bias5": bias5,
                "b_qg": bqg_col,
                "b_vvg": np.ascontiguousarray(
                    np.broadcast_to(bvvg[None], (128, HPC, 2, HD))
                ),
            }
        )
    return in_maps


def kernel(
    hidden_states,
    Wq,
    bq,
    Wk,
    bk,
    Wv,
    bv,
    Wqg,
    bqg,
    Wkg,
    bkg,
    Wvg,
    bvg,
    n_global,
):
    from concourse.bass_utils import run_bass_kernel_spmd

    assert int(n_global) == G
    nc = _get_program()
    in_maps = _prep_in_maps(
        hidden_states, Wq, bq, Wk, bk, Wv, bv, Wqg, bqg, Wkg, bkg, Wvg, bvg
    )
    res = run_bass_kernel_spmd(nc, in_maps, list(range(NCORES)))
    out = np.zeros((B, S, Dm), np.float32)
    for c in range(NCORES):
        b = c // 4
        cols = slice(HD * 3 * (c % 4), HD * (3 * (c % 4) + 3))
        out[b, :, cols] = res.results[c]["out"].T
    return out
